# revision 5
# baseline (speedup 1.0000x reference)
"""Trainium2 Bass kernel for nn_Encoder_5248450035714 (2-layer LSTM encoder).

x = emb[input_seq]; two LSTM layers; returns (h_n, c_n) each [2, B, H].
S=256, B=64, E=H=1024, vocab 32000.

Sharding: tensor-parallel over the 4H gate dim across 8 cores. Core c
owns 128 rows of each gate block (order i, f, o, g) => h-dims
[128c, 128c+128).

Structure (v2): ONE AllGather per tau carrying [h0(tau); h1(tau-2)].
Layer 1 lags layer 0 by two steps, so its input projection
xp1(s) = W_ih1 @ ys0(s) runs off the critical chain (computed at tau
s+1 from the gathered ys0). Startup (embedding gather -> x^T AllGather
-> input projection GEMM) is chunked; proj0 chunks c>=2 overlap the
recurrence, emitted as per-tau slices so the PE priority heap fills
idle time with them without delaying chain matmuls.
"""
import os
import sys

sys.path.insert(0, "/opt/trn_rl_repo")

import numpy as np
import ml_dtypes

BF16 = ml_dtypes.bfloat16

S, B, VOCAB, E, H = 256, 64, 32000, 1024, 1024
NCORES = 8
HC = H // NCORES          # 128 h-dims per core
TOK = S * B               # 16384 tokens
KE = E // 128             # 8 contraction chunks over E/H
GATE_ORDER = (0, 1, 3, 2)  # i, f, o, g (block index into the 4H dim)
NB = 4                    # hb ring depth
CH = 32                   # proj0/xp chunk: steps per chunk
NCHUNK = S // CH          # 8 chunks

_CACHE = {}


def _ensure_axon_hooks():
    try:
        import antenv
        if "/opt/trn_rl_repo/antenv" not in list(antenv.__path__):
            antenv.__path__.append("/opt/trn_rl_repo/antenv")
    except Exception:
        pass


def build_nc(n_steps=S):
    import concourse.bacc as bacc
    import concourse.mybir as mybir
    import concourse.tile as tile

    dt = mybir.dt
    AF = mybir.ActivationFunctionType
    nc = bacc.Bacc("TRN2", target_bir_lowering=False, debug=False,
                   num_devices=NCORES)
    ntok = n_steps * B
    ch = min(CH, n_steps)     # steps per chunk
    n_ch = max(1, n_steps // ch)
    ctok = ch * B             # tokens per chunk (2048)

    # ---- per-core inputs (host-sharded) ----
    tok = nc.dram_tensor("tok", [128, ntok // 16], dt.int16,
                         kind="ExternalInput")
    embc = nc.dram_tensor("embc", [VOCAB, 128], dt.bfloat16,
                          kind="ExternalInput")
    w_p0 = nc.dram_tensor("w_p0", [KE * 4 * 128, 128], dt.bfloat16,
                          kind="ExternalInput")   # proj0 lhsT tiles [k][m]
    w_r0 = nc.dram_tensor("w_r0", [KE * 4 * 128, 128], dt.bfloat16,
                          kind="ExternalInput")   # rec0 W_hh0^T tiles
    w_p1 = nc.dram_tensor("w_p1", [KE * 4 * 128, 128], dt.bfloat16,
                          kind="ExternalInput")   # W_ih1^T tiles
    w_r1 = nc.dram_tensor("w_r1", [KE * 4 * 128, 128], dt.bfloat16,
                          kind="ExternalInput")   # W_hh1^T tiles
    b0 = nc.dram_tensor("b0", [4 * 128, 1], dt.float32, kind="ExternalInput")
    b1 = nc.dram_tensor("b1", [4 * 128, 1], dt.float32, kind="ExternalInput")

    out = nc.dram_tensor("out", [4 * 128, B], dt.float32,
                         kind="ExternalOutput")

    xp0 = nc.dram_tensor("xp0", [4 * 128, ntok], dt.float32, kind="Internal")

    rg = [list(range(NCORES))]

    with tile.TileContext(nc) as tc:
        with tc.tile_pool(name="dram", bufs=1, space="DRAM") as dram, \
             tc.tile_pool(name="wpool", bufs=1) as wpool, \
             tc.tile_pool(name="gather", bufs=1) as gpool, \
             tc.tile_pool(name="xtiles", bufs=2) as xpool, \
             tc.tile_pool(name="psum", bufs=2, space="PSUM") as psum_pool, \
             tc.tile_pool(name="ew", bufs=3) as ewpool, \
             tc.tile_pool(name="state", bufs=1) as spool:

            def dma_blocks(dst2d, src, nblk, f):
                """DRAM [(n p), f] -> SBUF [p, (n f)] block-row layout."""
                return nc.sync.dma_start(
                    dst2d.rearrange("p (n f) -> p n f", f=f),
                    src.rearrange("(n p) f -> p n f", p=128))

            # ---- weights and biases ----
            w0_sb = wpool.tile([128, KE * 4 * 128], dt.bfloat16, tag="w0")
            dma_blocks(w0_sb[:], w_p0[:], KE * 4, 128)
            w0r_sb = wpool.tile([128, KE * 4 * 128], dt.bfloat16, tag="w0r")
            dma_blocks(w0r_sb[:], w_r0[:], KE * 4, 128)
            wp1_sb = wpool.tile([128, KE * 4 * 128], dt.bfloat16, tag="wp1")
            dma_blocks(wp1_sb[:], w_p1[:], KE * 4, 128)
            w1r_sb = wpool.tile([128, KE * 4 * 128], dt.bfloat16, tag="w1r")
            dma_blocks(w1r_sb[:], w_r1[:], KE * 4, 128)
            b0_sb = wpool.tile([128, 4], dt.float32, tag="b0")
            dma_blocks(b0_sb[:], b0[:], 4, 1)
            b1_sb = wpool.tile([128, 4], dt.float32, tag="b1")
            dma_blocks(b1_sb[:], b1[:], 4, 1)
            # bias1 broadcast across batch: [128, 4B]
            b1bc = wpool.tile([128, 4 * B], dt.float32, tag="b1bc")
            for m in range(4):
                nc.vector.tensor_copy(
                    b1bc[:, B * m:B * (m + 1)],
                    b1_sb[:, m:m + 1].to_broadcast([128, B]))

            # ============ Phase 1: gather x^T e-chunk ============
            idx_sb = gpool.tile([128, ntok // 16], dt.int16, tag="idx")
            nc.sync.dma_start(idx_sb[:], tok[:])
            xt_mine = gpool.tile([128, 1, ntok], dt.bfloat16, tag="xt")
            GCH = min(256, ntok)  # idxs per gather instruction
            for gi in range(ntok // GCH):
                nc.gpsimd.dma_gather(
                    xt_mine[:, :, GCH * gi:GCH * (gi + 1)],
                    embc[:],
                    idx_sb[:, (GCH // 16) * gi:(GCH // 16) * (gi + 1)],
                    num_idxs=GCH, num_idxs_reg=GCH, elem_size=128,
                    transpose=True,
                )

            # ============ Phase 2: AllGather x^T (chunked) ============
            agx_in = [dram.tile([128, ctok], dt.bfloat16, tag=f"agxi{c}",
                                name=f"agxi{c}")
                      for c in range(n_ch)]
            xtf = [dram.tile([NCORES * 128, ctok], dt.bfloat16,
                             tag=f"xtf{c}", name=f"xtf{c}",
                             addr_space="Shared")
                   for c in range(n_ch)]
            for c in range(n_ch):
                nc.sync.dma_start(agx_in[c][:],
                                  xt_mine[:, 0, ctok * c:ctok * (c + 1)])
                nc.gpsimd.collective_compute(
                    "AllGather", mybir.AluOpType.bypass,
                    ins=[agx_in[c].opt()], outs=[xtf[c].opt()],
                    replica_groups=rg)

            # ============ Phase 3: proj0 GEMM (chunk units) ============
            # one chunk = CH steps = ctok tokens; emitted as callables so
            # chunks >= 2 can be spliced into the recurrence trace.
            NT = min(512, ctok)

            def proj0_chunk_rhs(c):
                rhs = []
                for k in range(KE):
                    r = xpool.tile([128, ctok], dt.bfloat16, tag=f"rhs{k}",
                                   name=f"rhs{k}_{c}")
                    nc.sync.dma_start(
                        r[:], xtf[c][128 * k:128 * (k + 1), :])
                    rhs.append(r)
                return rhs

            def proj0_group(c, rhs, nt, m):
                """One (token-tile, gate) group of chunk c: 8 matmuls."""
                ps = psum_pool.tile([128, NT], dt.float32, tag="pj")
                for k in range(KE):
                    nc.tensor.matmul(
                        ps[:],
                        w0_sb[:, (k * 4 + m) * 128:(k * 4 + m + 1) * 128],
                        rhs[k][:, NT * nt:NT * (nt + 1)],
                        start=(k == 0), stop=(k == KE - 1))
                xo = ewpool.tile([128, NT], dt.float32, tag="xo")
                nc.scalar.activation(xo[:], ps[:], AF.Identity,
                                     bias=b0_sb[:, m:m + 1])
                nc.sync.dma_start(
                    xp0[128 * m:128 * (m + 1),
                        ctok * c + NT * nt:ctok * c + NT * (nt + 1)],
                    xo[:])

            # chunks 0,1 run up front (chunk 0 gates the first steps)
            n_up_front = min(2, n_ch)
            for c in range(n_up_front):
                rhs = proj0_chunk_rhs(c)
                for nt in range(ctok // NT):
                    for m in range(4):
                        proj0_group(c, rhs, nt, m)

            # remaining chunks: one (nt, m) group every other tau
            # schedule: chunk c must be done by tau 32*c; we have
            # 32*(c-1)..32*c window; ~17 work items per chunk, 32 taus
            # assign items to taus: chunk c's items into window
            # [CH*(c-2), CH*(c-1))
            tau_items = {t: [] for t in range(n_steps + 2)}
            for c in range(n_up_front, n_ch):
                items = [("rhs", c)] + [("grp", c, nt, m)
                                        for nt in range(ctok // NT)
                                        for m in range(4)]
                w0_t, w1_t = ch * (c - 2), ch * (c - 1)
                span = w1_t - w0_t
                for j, item in enumerate(items):
                    tau_items[w0_t + (j * span) // len(items)].append(item)

            chunk_rhs = {}

            def emit_tau_proj0(t):
                for item in tau_items.get(t, []):
                    if item[0] == "rhs":
                        chunk_rhs[item[1]] = proj0_chunk_rhs(item[1])
                    else:
                        _, c, nt, m = item
                        proj0_group(c, chunk_rhs[c], nt, m)

            # ============ Phase 4: recurrence ============
            # persistent state; gc0/gc1 hold [tanh(g) | c] per layer
            gc0 = spool.tile([128, 2 * B], dt.float32, tag="gc0")
            gc1 = spool.tile([128, 2 * B], dt.float32, tag="gc1")
            nc.vector.memset(gc0[:], 0.0)
            nc.vector.memset(gc1[:], 0.0)

            hb = [spool.tile([128, 2 * KE * B], dt.bfloat16,
                             tag=f"hb{i}", name=f"hb{i}")
                  for i in range(NB)]

            ys = [dram.tile([2 * NCORES * 128, B], dt.bfloat16,
                            tag=f"ys_{t}", name=f"ys_{t}",
                            addr_space="Shared")
                  for t in range(n_steps + 1)]
            agin = [dram.tile([256, B], dt.bfloat16, tag=f"agin{i}",
                              name=f"agin{i}")
                    for i in range(3)]

            sig, tnh = AF.Sigmoid, AF.Tanh

            # hb layout: block n of [128, B]; n = 2k   -> h0 chunk k
            #                                 n = 2k+1 -> h1 chunk k
            def h0_rhs(buf, k):
                return buf[:, B * 2 * k:B * (2 * k + 1)]

            def h1_rhs(buf, k):
                return buf[:, B * (2 * k + 1):B * (2 * k + 2)]

            def gates_mm(ps, w_sb, rhs_of, accumulate=False):
                for m in range(4):
                    for k in range(KE):
                        nc.tensor.matmul(
                            ps[:, B * m:B * (m + 1)],
                            w_sb[:, (k * 4 + m) * 128:(k * 4 + m + 1) * 128],
                            rhs_of(k),
                            start=(k == 0 and not accumulate),
                            stop=(k == KE - 1))

            def cell(g_sb, gc, agb_slice):
                """g_sb [128,4B] f32 pre-activations (i,f,o,g); gc [128,2B]
                holds [tanh(g) | c]. Writes h (bf16) into agb_slice."""
                nc.scalar.activation(g_sb[:, 0:3 * B], g_sb[:, 0:3 * B], sig)
                nc.scalar.activation(gc[:, 0:B], g_sb[:, 3 * B:4 * B], tnh)
                prod = ewpool.tile([128, 2 * B], dt.float32, tag="prod")
                nc.vector.tensor_mul(prod[:], g_sb[:, 0:2 * B], gc[:])
                nc.vector.tensor_add(gc[:, B:2 * B], prod[:, 0:B],
                                     prod[:, B:2 * B])
                tc_sb = ewpool.tile([128, B], dt.float32, tag="tc")
                nc.scalar.activation(tc_sb[:], gc[:, B:2 * B], tnh)
                nc.vector.tensor_mul(agb_slice, g_sb[:, 2 * B:3 * B],
                                     tc_sb[:])

            xp1_sb = [spool.tile([128, 4 * B], dt.float32, tag=f"xp1_{i}",
                                 name=f"xp1_{i}") for i in range(2)]

            h0_final = None
            h1_final = None
            last_tau = n_steps + 1
            for t in range(n_steps + 2):
                # ---- dma-in of AG(t-1) ----
                if 1 <= t <= n_steps + 1:
                    dma_blocks(hb[(t - 1) % NB][:], ys[t - 1][:], 2 * KE, B)

                emit_tau_proj0(t)

                agb_sb = ewpool.tile([128, 2 * B], dt.bfloat16,
                                     tag=f"agb{t % 3}", name=f"agb{t % 3}")

                # ---- L0 step t ----
                if t < n_steps:
                    xp_sb = ewpool.tile([128, 4 * B], dt.float32,
                                        tag=f"xp{t % 3}")
                    dma_blocks(xp_sb[:], xp0[:, B * t:B * (t + 1)], 4, B)
                    if t == 0:
                        g0 = xp_sb
                    else:
                        ps0 = psum_pool.tile([128, 4 * B], dt.float32,
                                             tag="ps0")
                        gates_mm(ps0, w0r_sb,
                                 lambda k: h0_rhs(hb[(t - 1) % NB], k))
                        g0 = ewpool.tile([128, 4 * B], dt.float32, tag="g0")
                        nc.vector.tensor_add(g0[:], ps0[:], xp_sb[:])
                    cell(g0, gc0, agb_sb[:, 0:B])
                    if t == n_steps - 1:
                        h0_final = agb_sb
                else:
                    nc.vector.memset(agb_sb[:, 0:B], 0.0)

                # ---- L1 step t-2 (chain) ----
                s1 = t - 2
                if 0 <= s1 < n_steps:
                    g1 = ewpool.tile([128, 4 * B], dt.float32, tag="g1")
                    if s1 == 0:
                        nc.vector.tensor_copy(g1[:], xp1_sb[s1 % 2][:])
                    else:
                        ps1 = psum_pool.tile([128, 4 * B], dt.float32,
                                             tag="ps1")
                        gates_mm(ps1, w1r_sb,
                                 lambda k: h1_rhs(hb[(s1 + 1) % NB], k))
                        nc.vector.tensor_add(g1[:], ps1[:],
                                             xp1_sb[s1 % 2][:])
                    cell(g1, gc1, agb_sb[:, B:2 * B])
                    if s1 == n_steps - 1:
                        h1_final = agb_sb
                else:
                    nc.vector.memset(agb_sb[:, B:2 * B], 0.0)

                # ---- xp1 for step t-1 (off-chain, fills PE idle) ----
                s = t - 1
                if 0 <= s < n_steps:
                    psx = psum_pool.tile([128, 4 * B], dt.float32,
                                         tag="psx")
                    gates_mm(psx, wp1_sb,
                             lambda k: h0_rhs(hb[s % NB], k))
                    nc.vector.tensor_add(xp1_sb[s % 2][:], psx[:], b1bc[:])

                # ---- combined AllGather ----
                if t <= n_steps:
                    nc.sync.dma_start(
                        agin[t % 3].rearrange("(n p) f -> p n f", p=128),
                        agb_sb.rearrange("p (n f) -> p n f", f=B))
                    nc.gpsimd.collective_compute(
                        "AllGather", mybir.AluOpType.bypass,
                        ins=[agin[t % 3].opt()], outs=[ys[t].opt()],
                        replica_groups=rg)

            # ---- outputs ----
            of = ewpool.tile([128, B], dt.float32, tag="of")
            nc.scalar.activation(of[:], h0_final[:, 0:B], AF.Copy)
            nc.sync.dma_start(out[0:128, :], of[:])
            nc.sync.dma_start(out[128:256, :], gc0[:, B:2 * B])
            of2 = ewpool.tile([128, B], dt.float32, tag="of2")
            nc.scalar.activation(of2[:], h1_final[:, B:2 * B], AF.Copy)
            nc.sync.dma_start(out[256:384, :], of2[:])
            nc.sync.dma_start(out[384:512, :], gc1[:, B:2 * B])

    nc.compile()
    return nc


def _host_prep(inputs, n_steps=S):
    """Build per-core in_maps from full inputs."""
    seq = np.asarray(inputs["input_seq"])[:n_steps].astype(np.int64)
    emb = np.asarray(inputs["emb"], dtype=np.float32)
    ntok = n_steps * B

    toks = seq.reshape(-1).astype(np.int16)  # vocab < 32768
    wrapped = toks.reshape(ntok // 16, 16).T.copy()       # [16, ntok/16]
    wrapped128 = np.tile(wrapped, (8, 1)).astype(np.int16)  # [128, ntok/16]

    w_ih_0T = np.asarray(inputs["w_ih_0"], np.float32).T
    w_hh_0T = np.asarray(inputs["w_hh_0"], np.float32).T
    w_ih_1T = np.asarray(inputs["w_ih_1"], np.float32).T
    w_hh_1T = np.asarray(inputs["w_hh_1"], np.float32).T
    b0sum = (np.asarray(inputs["b_ih_0"], np.float32) +
             np.asarray(inputs["b_hh_0"], np.float32))
    b1sum = (np.asarray(inputs["b_ih_1"], np.float32) +
             np.asarray(inputs["b_hh_1"], np.float32))

    in_maps = []
    for c in range(NCORES):
        m = {"tok": wrapped128,
             "embc": emb[:, 128 * c:128 * (c + 1)].astype(BF16)}

        def tiles(wT, nk):
            cols = np.concatenate(
                [wT[:, H * gb + HC * c: H * gb + HC * (c + 1)]
                 for gb in GATE_ORDER], axis=1)  # [K, 512]
            arr = np.zeros((nk * 4 * 128, 128), dtype=BF16)
            for k in range(nk):
                for mm in range(4):
                    arr[(k * 4 + mm) * 128:(k * 4 + mm + 1) * 128] = \
                        cols[128 * k:128 * (k + 1),
                             128 * mm:128 * (mm + 1)].astype(BF16)
            return arr

        m["w_p0"] = tiles(w_ih_0T, KE)
        m["w_r0"] = tiles(w_hh_0T, KE)
        m["w_p1"] = tiles(w_ih_1T, KE)
        m["w_r1"] = tiles(w_hh_1T, KE)

        def bias(bsum):
            v = np.concatenate(
                [bsum[H * gb + HC * c: H * gb + HC * (c + 1)]
                 for gb in GATE_ORDER])
            return v.reshape(4 * 128, 1).astype(np.float32)

        m["b0"] = bias(b0sum)
        m["b1"] = bias(b1sum)
        in_maps.append(m)
    return in_maps


def _assemble(results):
    h_n = np.zeros((2, B, H), np.float32)
    c_n = np.zeros((2, B, H), np.float32)
    for c in range(NCORES):
        o = results[c]["out"]
        h_n[0][:, HC * c:HC * (c + 1)] = o[0:128].T
        c_n[0][:, HC * c:HC * (c + 1)] = o[128:256].T
        h_n[1][:, HC * c:HC * (c + 1)] = o[256:384].T
        c_n[1][:, HC * c:HC * (c + 1)] = o[384:512].T
    return h_n, c_n


def run_on_hw(inputs, n_steps=S, trace=False):
    _ensure_axon_hooks()
    from concourse.bass_utils import run_bass_kernel_spmd
    if n_steps not in _CACHE:
        _CACHE[n_steps] = build_nc(n_steps)
    nc = _CACHE[n_steps]
    in_maps = _host_prep(inputs, n_steps)
    res = run_bass_kernel_spmd(nc, in_maps, core_ids=list(range(NCORES)),
                               trace=trace)
    h_n, c_n = _assemble(res.results)
    return (h_n, c_n), res


def kernel(**inputs):
    (h_n, c_n), _ = run_on_hw(inputs, S, trace=False)
    return (h_n, c_n)


if __name__ == "__main__":
    ns = int(os.environ.get("NSTEPS", "4"))
    build_nc(ns)
    print("build OK", ns)


# revision 11
# speedup vs baseline: 1.0521x; 1.0521x over previous
"""Trainium2 Bass kernel for nn_Encoder_5248450035714 (2-layer LSTM encoder).

x = emb[input_seq]; two LSTM layers; returns (h_n, c_n) each [2, B, H].
S=256, B=64, E=H=1024, vocab 32000.

Sharding: tensor-parallel over the 4H gate dim across 8 cores. Core c
owns 128 rows of each gate block (order i, f, o, g) => h-dims
[128c, 128c+128).

Structure (v2): ONE AllGather per tau carrying [h0(tau); h1(tau-2)].
Layer 1 lags layer 0 by two steps, so its input projection
xp1(s) = W_ih1 @ ys0(s) runs off the critical chain (computed at tau
s+1 from the gathered ys0). Startup (embedding gather -> x^T AllGather
-> input projection GEMM) is chunked; proj0 chunks c>=2 overlap the
recurrence, emitted as per-tau slices so the PE priority heap fills
idle time with them without delaying chain matmuls.
"""
import os
import sys

sys.path.insert(0, "/opt/trn_rl_repo")

import numpy as np
import ml_dtypes

BF16 = ml_dtypes.bfloat16
E4 = ml_dtypes.float8_e4m3
FP8 = os.environ.get("FP8", "0") == "1"

S, B, VOCAB, E, H = 256, 64, 32000, 1024, 1024
NCORES = 8
HC = H // NCORES          # 128 h-dims per core
TOK = S * B               # 16384 tokens
KE = E // 128             # 8 contraction chunks over E/H
GATE_ORDER = (0, 1, 3, 2)  # i, f, o, g (block index into the 4H dim)
NB = 4                    # hb ring depth
CH = 32                   # proj0/xp chunk: steps per chunk
NCHUNK = S // CH          # 8 chunks

_CACHE = {}


def _ensure_axon_hooks():
    try:
        import antenv
        if "/opt/trn_rl_repo/antenv" not in list(antenv.__path__):
            antenv.__path__.append("/opt/trn_rl_repo/antenv")
    except Exception:
        pass


def build_nc(n_steps=S):
    import concourse.bacc as bacc
    import concourse.mybir as mybir
    import concourse.tile as tile

    dt = mybir.dt
    AF = mybir.ActivationFunctionType
    nc = bacc.Bacc("TRN2", target_bir_lowering=False, debug=False,
                   num_devices=NCORES)
    ntok = n_steps * B
    ch = min(CH, n_steps)     # steps per chunk
    n_ch = max(1, n_steps // ch)
    ctok = ch * B             # tokens per chunk (2048)

    # ---- per-core inputs (host-sharded) ----
    tok = nc.dram_tensor("tok", [128, ntok // 16], dt.int16,
                         kind="ExternalInput")
    embc = nc.dram_tensor("embc", [VOCAB, 128], dt.bfloat16,
                          kind="ExternalInput")
    w_p0 = nc.dram_tensor("w_p0", [KE * 4 * 128, 128], dt.bfloat16,
                          kind="ExternalInput")   # proj0 lhsT tiles [k][m]
    rdt = dt.float8e4 if FP8 else dt.bfloat16
    w_r0 = nc.dram_tensor("w_r0", [KE * 4 * 128, 128], rdt,
                          kind="ExternalInput")   # rec0 W_hh0^T tiles
    w_p1 = nc.dram_tensor("w_p1", [KE * 4 * 128, 128], rdt,
                          kind="ExternalInput")   # W_ih1^T tiles
    w_r1 = nc.dram_tensor("w_r1", [KE * 4 * 128, 128], rdt,
                          kind="ExternalInput")   # W_hh1^T tiles
    b0 = nc.dram_tensor("b0", [4 * 128, 1], dt.float32, kind="ExternalInput")
    b1 = nc.dram_tensor("b1", [4 * 128, 1], dt.float32, kind="ExternalInput")

    out = nc.dram_tensor("out", [4 * 128, B], dt.float32,
                         kind="ExternalOutput")

    xp0 = nc.dram_tensor("xp0", [4 * 128, ntok], dt.float32, kind="Internal")

    rg = [list(range(NCORES))]

    with tile.TileContext(nc) as tc:
        with tc.tile_pool(name="dram", bufs=1, space="DRAM") as dram, \
             tc.tile_pool(name="wpool", bufs=1) as wpool, \
             tc.tile_pool(name="gather", bufs=1) as gpool, \
             tc.tile_pool(name="xtiles", bufs=2) as xpool, \
             tc.tile_pool(name="psum", bufs=2, space="PSUM") as psum_pool, \
             tc.tile_pool(name="ew", bufs=3) as ewpool, \
             tc.tile_pool(name="state", bufs=1) as spool:

            def dma_blocks(dst2d, src, nblk, f):
                """DRAM [(n p), f] -> SBUF [p, (n f)] block-row layout."""
                return nc.sync.dma_start(
                    dst2d.rearrange("p (n f) -> p n f", f=f),
                    src.rearrange("(n p) f -> p n f", p=128))

            # ---- weights and biases ----
            w0_sb = wpool.tile([128, KE * 4 * 128], dt.bfloat16, tag="w0")
            dma_blocks(w0_sb[:], w_p0[:], KE * 4, 128)
            w0r_sb = wpool.tile([128, KE * 4 * 128], rdt, tag="w0r")
            dma_blocks(w0r_sb[:], w_r0[:], KE * 4, 128)
            wp1_sb = wpool.tile([128, KE * 4 * 128], rdt, tag="wp1")
            dma_blocks(wp1_sb[:], w_p1[:], KE * 4, 128)
            w1r_sb = wpool.tile([128, KE * 4 * 128], rdt, tag="w1r")
            dma_blocks(w1r_sb[:], w_r1[:], KE * 4, 128)
            b0_sb = wpool.tile([128, 4], dt.float32, tag="b0")
            dma_blocks(b0_sb[:], b0[:], 4, 1)
            b1_sb = wpool.tile([128, 4], dt.float32, tag="b1")
            dma_blocks(b1_sb[:], b1[:], 4, 1)
            # bias1 broadcast across batch: [128, 4B]
            b1bc = wpool.tile([128, 4 * B], dt.float32, tag="b1bc")
            for m in range(4):
                nc.vector.tensor_copy(
                    b1bc[:, B * m:B * (m + 1)],
                    b1_sb[:, m:m + 1].to_broadcast([128, B]))

            # ============ Phase 1: gather x^T e-chunk ============
            idx_sb = gpool.tile([128, ntok // 16], dt.int16, tag="idx")
            nc.sync.dma_start(idx_sb[:], tok[:])
            xt_mine = gpool.tile([128, 1, ntok], dt.bfloat16, tag="xt")
            GCH = min(256, ntok)  # idxs per gather instruction
            for gi in range(ntok // GCH):
                nc.gpsimd.dma_gather(
                    xt_mine[:, :, GCH * gi:GCH * (gi + 1)],
                    embc[:],
                    idx_sb[:, (GCH // 16) * gi:(GCH // 16) * (gi + 1)],
                    num_idxs=GCH, num_idxs_reg=GCH, elem_size=128,
                    transpose=True,
                )

            # ============ Phase 2: AllGather x^T (1MB chunks) ============
            xch = min(2 * ch, n_steps)        # steps per AGx chunk (64)
            xctok = xch * B
            n_xch = max(1, ntok // xctok)
            agx_in = [dram.tile([128, xctok], dt.bfloat16, tag=f"agxi{c}",
                                name=f"agxi{c}")
                      for c in range(n_xch)]
            xtf = [dram.tile([NCORES * 128, xctok], dt.bfloat16,
                             tag=f"xtf{c}", name=f"xtf{c}",
                             addr_space="Shared")
                   for c in range(n_xch)]
            for c in range(n_xch):
                nc.sync.dma_start(agx_in[c][:],
                                  xt_mine[:, 0, xctok * c:xctok * (c + 1)])
                nc.gpsimd.collective_compute(
                    "AllGather", mybir.AluOpType.bypass,
                    ins=[agx_in[c].opt()], outs=[xtf[c].opt()],
                    replica_groups=rg)

            # ============ Phase 3: proj0 GEMM (chunk units) ============
            # one chunk = CH steps = ctok tokens; emitted as callables so
            # chunks >= 2 can be spliced into the recurrence trace.
            NT = min(512, ctok)

            def proj0_chunk_rhs(c):
                xc, col0 = (c * ctok) // xctok, (c * ctok) % xctok
                rhs = []
                for k in range(KE):
                    r = xpool.tile([128, ctok], dt.bfloat16, tag=f"rhs{k}",
                                   name=f"rhs{k}_{c}")
                    nc.sync.dma_start(
                        r[:], xtf[xc][128 * k:128 * (k + 1),
                                      col0:col0 + ctok])
                    rhs.append(r)
                return rhs

            def proj0_group(c, rhs, nt, m):
                """One (token-tile, gate) group of chunk c: 8 matmuls."""
                ps = psum_pool.tile([128, NT], dt.float32, tag="pj")
                for k in range(KE):
                    nc.tensor.matmul(
                        ps[:],
                        w0_sb[:, (k * 4 + m) * 128:(k * 4 + m + 1) * 128],
                        rhs[k][:, NT * nt:NT * (nt + 1)],
                        start=(k == 0), stop=(k == KE - 1))
                xo = ewpool.tile([128, NT], dt.float32, tag="xo")
                nc.scalar.activation(xo[:], ps[:], AF.Identity,
                                     bias=b0_sb[:, m:m + 1])
                nc.sync.dma_start(
                    xp0[128 * m:128 * (m + 1),
                        ctok * c + NT * nt:ctok * c + NT * (nt + 1)],
                    xo[:])

            # chunks 0,1 run up front (chunk 0 gates the first steps)
            n_up_front = min(2, n_ch)
            for c in range(n_up_front):
                rhs = proj0_chunk_rhs(c)
                for nt in range(ctok // NT):
                    for m in range(4):
                        proj0_group(c, rhs, nt, m)

            # remaining chunks: one (nt, m) group every other tau
            # schedule: chunk c must be done by tau 32*c; we have
            # 32*(c-1)..32*c window; ~17 work items per chunk, 32 taus
            # assign items to taus: chunk c's items into window
            # [CH*(c-2), CH*(c-1))
            tau_items = {t: [] for t in range(n_steps + 2)}
            for c in range(n_up_front, n_ch):
                items = [("rhs", c)] + [("grp", c, nt, m)
                                        for nt in range(ctok // NT)
                                        for m in range(4)]
                w0_t, w1_t = ch * (c - 2), ch * (c - 1)
                span = w1_t - w0_t
                for j, item in enumerate(items):
                    tau_items[w0_t + (j * span) // len(items)].append(item)

            chunk_rhs = {}

            def emit_tau_proj0(t):
                for item in tau_items.get(t, []):
                    if item[0] == "rhs":
                        chunk_rhs[item[1]] = proj0_chunk_rhs(item[1])
                    else:
                        _, c, nt, m = item
                        proj0_group(c, chunk_rhs[c], nt, m)

            # ============ Phase 4: recurrence ============
            # persistent state; gc0/gc1 hold [tanh(g) | c] per layer
            gc0 = spool.tile([128, 2 * B], dt.float32, tag="gc0")
            gc1 = spool.tile([128, 2 * B], dt.float32, tag="gc1")
            nc.vector.memset(gc0[:], 0.0)
            nc.vector.memset(gc1[:], 0.0)

            hb = [spool.tile([128, 2 * KE * B], rdt,
                             tag=f"hb{i}", name=f"hb{i}")
                  for i in range(NB)]

            ys = [dram.tile([2 * NCORES * 128, B], rdt,
                            tag=f"ys_{t}", name=f"ys_{t}",
                            addr_space="Shared")
                  for t in range(n_steps + 1)]
            agin = [dram.tile([256, B], rdt, tag=f"agin{i}",
                              name=f"agin{i}")
                    for i in range(3)]

            sig, tnh = AF.Sigmoid, AF.Tanh

            # hb layout: block n of [128, B]; n = 2k   -> h0 chunk k
            #                                 n = 2k+1 -> h1 chunk k
            def h0_rhs(buf, k):
                return buf[:, B * 2 * k:B * (2 * k + 1)]

            def h1_rhs(buf, k):
                return buf[:, B * (2 * k + 1):B * (2 * k + 2)]

            def gates_mm(ps, w_sb, rhs_of, accumulate=False):
                for m in range(4):
                    for k in range(KE):
                        nc.tensor.matmul(
                            ps[:, B * m:B * (m + 1)],
                            w_sb[:, (k * 4 + m) * 128:(k * 4 + m + 1) * 128],
                            rhs_of(k),
                            start=(k == 0 and not accumulate),
                            stop=(k == KE - 1))

            def cell(g_sb, gc, agb_slice, h32=None):
                """g_sb [128,4B] f32 pre-activations (i,f,o,g); gc [128,2B]
                holds [tanh(g) | c]. Writes h into agb_slice."""
                nc.scalar.activation(g_sb[:, 0:3 * B], g_sb[:, 0:3 * B], sig)
                nc.scalar.activation(gc[:, 0:B], g_sb[:, 3 * B:4 * B], tnh)
                prod = ewpool.tile([128, 2 * B], dt.float32, tag="prod")
                nc.vector.tensor_mul(prod[:], g_sb[:, 0:2 * B], gc[:])
                nc.vector.tensor_add(gc[:, B:2 * B], prod[:, 0:B],
                                     prod[:, B:2 * B])
                tc_sb = ewpool.tile([128, B], dt.float32, tag="tc")
                nc.scalar.activation(tc_sb[:], gc[:, B:2 * B], tnh)
                nc.vector.tensor_mul(agb_slice, g_sb[:, 2 * B:3 * B],
                                     tc_sb[:])
                if h32 is not None:
                    nc.vector.tensor_mul(h32, g_sb[:, 2 * B:3 * B],
                                         tc_sb[:])

            xp1_sb = [spool.tile([128, 4 * B], dt.float32, tag=f"xp1_{i}",
                                 name=f"xp1_{i}") for i in range(2)]

            h0f32 = spool.tile([128, B], dt.float32, tag="h0f32")
            h1f32 = spool.tile([128, B], dt.float32, tag="h1f32")
            h0_final = None
            h1_final = None
            last_tau = n_steps + 1
            for t in range(n_steps + 2):
                # ---- dma-in of AG(t-1): h0 and h1 halves on the two
                # HWDGE rings in parallel; L0's matmuls only wait on h0
                if 1 <= t <= n_steps + 1:
                    hbt = hb[(t - 1) % NB]
                    dst = hbt.rearrange("p (n q f) -> p n q f", q=2, f=B)
                    src = ys[t - 1].rearrange("(n q p) f -> p n q f",
                                              p=128, q=2)
                    nc.sync.dma_start(dst[:, :, 0, :], src[:, :, 0, :])
                    nc.scalar.dma_start(dst[:, :, 1, :], src[:, :, 1, :])

                agb_sb = ewpool.tile([128, 2 * B], rdt,
                                     tag=f"agb{t % 3}", name=f"agb{t % 3}")

                # ---- L0 step t ----
                if t < n_steps:
                    xp_sb = ewpool.tile([128, 4 * B], dt.float32,
                                        tag=f"xp{t % 3}")
                    dma_blocks(xp_sb[:], xp0[:, B * t:B * (t + 1)], 4, B)
                    if t == 0:
                        g0 = xp_sb
                    else:
                        ps0 = psum_pool.tile([128, 4 * B], dt.float32,
                                             tag="ps0")
                        gates_mm(ps0, w0r_sb,
                                 lambda k: h0_rhs(hb[(t - 1) % NB], k))
                        g0 = ewpool.tile([128, 4 * B], dt.float32, tag="g0")
                        nc.vector.tensor_add(g0[:], ps0[:], xp_sb[:])
                    cell(g0, gc0, agb_sb[:, 0:B],
                         h32=(h0f32[:] if t == n_steps - 1 else None))
                    if t == n_steps - 1:
                        h0_final = agb_sb
                else:
                    nc.vector.memset(agb_sb[:, 0:B], 0.0)

                # ---- L1 step t-2 (chain) ----
                s1 = t - 2
                if 0 <= s1 < n_steps:
                    g1 = ewpool.tile([128, 4 * B], dt.float32, tag="g1")
                    if s1 == 0:
                        nc.vector.tensor_copy(g1[:], xp1_sb[s1 % 2][:])
                    else:
                        ps1 = psum_pool.tile([128, 4 * B], dt.float32,
                                             tag="ps1")
                        gates_mm(ps1, w1r_sb,
                                 lambda k: h1_rhs(hb[(s1 + 1) % NB], k))
                        nc.vector.tensor_add(g1[:], ps1[:],
                                             xp1_sb[s1 % 2][:])
                    cell(g1, gc1, agb_sb[:, B:2 * B],
                         h32=(h1f32[:] if s1 == n_steps - 1 else None))
                    if s1 == n_steps - 1:
                        h1_final = agb_sb
                else:
                    nc.vector.memset(agb_sb[:, B:2 * B], 0.0)

                # ---- xp1 for step t-1 (off-chain, fills PE idle) ----
                s = t - 1
                if 0 <= s < n_steps:
                    psx = psum_pool.tile([128, 4 * B], dt.float32,
                                         tag="psx")
                    gates_mm(psx, wp1_sb,
                             lambda k: h0_rhs(hb[s % NB], k))
                    nc.vector.tensor_add(xp1_sb[s % 2][:], psx[:], b1bc[:])

                # ---- combined AllGather ----
                if t <= n_steps:
                    nc.sync.dma_start(
                        agin[t % 3].rearrange("(n p) f -> p n f", p=128),
                        agb_sb.rearrange("p (n f) -> p n f", f=B))
                    nc.gpsimd.collective_compute(
                        "AllGather", mybir.AluOpType.bypass,
                        ins=[agin[t % 3].opt()], outs=[ys[t].opt()],
                        replica_groups=rg)

                # proj0 filler last: lowest priority within this tau
                emit_tau_proj0(t)

            # ---- outputs ----
            nc.sync.dma_start(out[0:128, :], h0f32[:])
            nc.sync.dma_start(out[128:256, :], gc0[:, B:2 * B])
            nc.sync.dma_start(out[256:384, :], h1f32[:])
            nc.sync.dma_start(out[384:512, :], gc1[:, B:2 * B])

    nc.compile()
    return nc


def _host_prep(inputs, n_steps=S):
    """Build per-core in_maps from full inputs."""
    seq = np.asarray(inputs["input_seq"])[:n_steps].astype(np.int64)
    emb = np.asarray(inputs["emb"], dtype=np.float32)
    ntok = n_steps * B

    toks = seq.reshape(-1).astype(np.int16)  # vocab < 32768
    wrapped = toks.reshape(ntok // 16, 16).T.copy()       # [16, ntok/16]
    wrapped128 = np.tile(wrapped, (8, 1)).astype(np.int16)  # [128, ntok/16]

    w_ih_0T = np.asarray(inputs["w_ih_0"], np.float32).T
    w_hh_0T = np.asarray(inputs["w_hh_0"], np.float32).T
    w_ih_1T = np.asarray(inputs["w_ih_1"], np.float32).T
    w_hh_1T = np.asarray(inputs["w_hh_1"], np.float32).T
    b0sum = (np.asarray(inputs["b_ih_0"], np.float32) +
             np.asarray(inputs["b_hh_0"], np.float32))
    b1sum = (np.asarray(inputs["b_ih_1"], np.float32) +
             np.asarray(inputs["b_hh_1"], np.float32))

    in_maps = []
    for c in range(NCORES):
        m = {"tok": wrapped128,
             "embc": emb[:, 128 * c:128 * (c + 1)].astype(BF16)}

        def tiles(wT, nk, tdt=BF16):
            cols = np.concatenate(
                [wT[:, H * gb + HC * c: H * gb + HC * (c + 1)]
                 for gb in GATE_ORDER], axis=1)  # [K, 512]
            arr = np.zeros((nk * 4 * 128, 128), dtype=tdt)
            for k in range(nk):
                for mm in range(4):
                    arr[(k * 4 + mm) * 128:(k * 4 + mm + 1) * 128] = \
                        cols[128 * k:128 * (k + 1),
                             128 * mm:128 * (mm + 1)].astype(tdt)
            return arr

        rdt_np = E4 if FP8 else BF16
        m["w_p0"] = tiles(w_ih_0T, KE)
        m["w_r0"] = tiles(w_hh_0T, KE, rdt_np)
        m["w_p1"] = tiles(w_ih_1T, KE, rdt_np)
        m["w_r1"] = tiles(w_hh_1T, KE, rdt_np)

        def bias(bsum):
            v = np.concatenate(
                [bsum[H * gb + HC * c: H * gb + HC * (c + 1)]
                 for gb in GATE_ORDER])
            return v.reshape(4 * 128, 1).astype(np.float32)

        m["b0"] = bias(b0sum)
        m["b1"] = bias(b1sum)
        in_maps.append(m)
    return in_maps


def _assemble(results):
    h_n = np.zeros((2, B, H), np.float32)
    c_n = np.zeros((2, B, H), np.float32)
    for c in range(NCORES):
        o = results[c]["out"]
        h_n[0][:, HC * c:HC * (c + 1)] = o[0:128].T
        c_n[0][:, HC * c:HC * (c + 1)] = o[128:256].T
        h_n[1][:, HC * c:HC * (c + 1)] = o[256:384].T
        c_n[1][:, HC * c:HC * (c + 1)] = o[384:512].T
    return h_n, c_n


def run_on_hw(inputs, n_steps=S, trace=False):
    _ensure_axon_hooks()
    from concourse.bass_utils import run_bass_kernel_spmd
    if n_steps not in _CACHE:
        _CACHE[n_steps] = build_nc(n_steps)
    nc = _CACHE[n_steps]
    in_maps = _host_prep(inputs, n_steps)
    res = run_bass_kernel_spmd(nc, in_maps, core_ids=list(range(NCORES)),
                               trace=trace)
    h_n, c_n = _assemble(res.results)
    return (h_n, c_n), res


def kernel(**inputs):
    (h_n, c_n), _ = run_on_hw(inputs, S, trace=False)
    return (h_n, c_n)


if __name__ == "__main__":
    ns = int(os.environ.get("NSTEPS", "4"))
    build_nc(ns)
    print("build OK", ns)
